# revision 28
# baseline (speedup 1.0000x reference)
"""Trainium2 Bass kernel for nn_CvtNodeInitializer (gnn_message_passing).

Strategy (per the sharding hint: partition nodes, route edges by tail-node
owner, replicate the projection weight):
  - Host: filter edges whose tail is a CVT node (only those contribute),
    sort by tail, and route each edge's feature rows to the core owning
    the tail. CVT nodes are compacted and greedily packed into windows of
    <=128 nodes AND <=128 edge slots, so each window is one PSUM tile and
    one 128-slot edge tile. Non-CVT rows never touch the device: the host
    scatters the computed CVT rows into a copy of node_tokens.
  - Device (SPMD, identical on 8 cores): per window, one fused matmul
    X^T-chunks @ [W_msg.T | a_eff] produces msg AND the per-edge logits
    (a_eff = attn @ W_msg folds the attention dot into the projection).
    The scalar engine exponentiates the logit column and then copies msg
    to SBUF scaled by q = exp(logit) per edge. The 0/1 one-hot seg matrix
    is host-precomputed (it doesn't depend on q) and rides in the same
    DMA as X. One matmul OH.T @ [q*msg | q] segment-reduces agg and den;
    DVE stages the PSUM result to SBUF as bf16 for the grouped store.
    Host normalizes (agg/den + shared) while scattering.
  - X, W, OH travel as bf16 (halves HBM traffic; matmuls at 1 cyc/row).
    DMA is grouped G windows per transfer to amortize descriptor-gen cost.
"""

import sys

sys.path.insert(0, "/opt/trn_rl_repo")

import numpy as np
import ml_dtypes

N_NODES = 200000
N_EDGES = 200000
HID = 256
NCORES = 8
P = 128
G = 8           # windows per DMA group
WAUG = 257      # msg cols (256) + logit/den col
CHW = 5         # per-window xg chunks: 4 X^T chunks + 1 one-hot chunk

_PROGRAM_CACHE: dict = {}


def _build_program(W: int, repeats: int = 1):
    """Per-core Bass program. W = windows per core (multiple of G)."""
    import concourse.bacc as bacc
    import concourse.mybir as mybir
    import concourse.tile as tile

    f32 = mybir.dt.float32
    bf16 = mybir.dt.bfloat16
    Act = mybir.ActivationFunctionType

    assert W % G == 0
    WG = W // G

    nc = bacc.Bacc()
    xt = nc.declare_dram_parameter("xt", [WG, P, G * CHW * P], bf16,
                                   isOutput=False)
    wch = nc.declare_dram_parameter("wch", [P, 4 * WAUG], bf16, isOutput=False)
    out = nc.declare_dram_parameter("out", [WG, P, G * WAUG], bf16,
                                    isOutput=True)

    with tile.TileContext(nc) as tc:
        with (
            tc.tile_pool(name="const", bufs=1) as cpool,
            tc.tile_pool(name="x", bufs=3) as xpool,
            tc.tile_pool(name="og", bufs=2) as ogpool,
            tc.tile_pool(name="msg", bufs=3) as mpool,
            tc.tile_pool(name="small", bufs=4) as spool,
            tc.tile_pool(name="pmsg", bufs=4, space="PSUM") as pmpool,
            tc.tile_pool(name="pagg", bufs=3, space="PSUM") as papool,
        ):
            # --- one-time constants ---
            wtile = cpool.tile([P, 4 * WAUG], bf16)
            warm = cpool.tile([P, 1], f32)
            nc.sync.dma_start(out=wtile[:], in_=wch[:])
            # hoist the Copy/Exp act-table load out of the repeat loop
            nc.scalar.activation(warm[:, 0:1], wtile[:, 0:1], Act.Exp)

            def stage_front(w, xg):
                """msg+logit matmul, exp, q-scaled msg copy — for window w."""
                k = w % G
                pm = pmpool.tile([P, WAUG], f32, tag="pm")
                for c in range(4):
                    nc.tensor.matmul(
                        pm[:, :],
                        lhsT=xg[:, (k * CHW + c) * P:(k * CHW + c + 1) * P],
                        rhs=wtile[:, c * WAUG:(c + 1) * WAUG],
                        start=(c == 0),
                        stop=(c == 3),
                    )
                # msgb col 256 = q = exp(logit); cols 0:256 = q * msg
                qt = spool.tile([P, 1], f32, tag="qt")
                nc.scalar.activation(qt[:, 0:1], pm[:, HID:WAUG], Act.Exp)
                msgb = mpool.tile([P, WAUG], bf16, tag="msgb")
                nc.scalar.activation(msgb[:, 0:HID], pm[:, 0:HID], Act.Copy,
                                     scale=qt[:, 0:1])
                nc.vector.tensor_copy(msgb[:, HID:WAUG], qt[:, 0:1])
                return msgb

            def stage_back(w, msgb, oh, og):
                """segment-reduce [agg | den] and stage for store."""
                k = w % G
                pa = papool.tile([P, WAUG], f32, tag="pa")
                nc.tensor.matmul(pa[:, :], lhsT=oh, rhs=msgb[:, :],
                                 start=True, stop=True)
                nc.vector.tensor_copy(og[:, k * WAUG:(k + 1) * WAUG], pa[:, :])

            LAG = 2

            def all_windows():
                pending = []  # [(w, msgb, oh, og), ...]

                def drain_one():
                    item = pending.pop(0)
                    stage_back(*item)
                    if item[0] % G == G - 1:
                        nc.sync.dma_start(out=out[item[0] // G], in_=item[3])

                for g in range(WG):
                    xg = xpool.tile([P, G * CHW * P], bf16, tag="xg")
                    og = ogpool.tile([P, G * WAUG], bf16, tag="og")
                    nc.sync.dma_start(out=xg[:], in_=xt[g])
                    for k in range(G):
                        w = g * G + k
                        msgb = stage_front(w, xg)
                        oh = xg[:, (k * CHW + 4) * P:(k * CHW + 5) * P]
                        pending.append((w, msgb, oh, og))
                        if len(pending) > LAG:
                            drain_one()
                while pending:
                    drain_one()

            if repeats == 1:
                all_windows()
            else:
                with tc.For_i(0, repeats, 1) as _iv:
                    all_windows()

    nc.compile()
    return nc


def _host_prep(node_tokens, relation_tokens, edge_index, node_is_cvt,
               shared_cvt, attn_vector, W_msg, n_cores=NCORES):
    """Routing + per-core input construction.

    Returns (in_maps, W, scatter) where scatter = (node_ids, flat_rows)
    per core: out_full[node_ids] = dev_out[flat_rows]."""
    node_tokens = np.asarray(node_tokens, np.float32)
    relation_tokens = np.asarray(relation_tokens, np.float32)
    n_nodes, hid = node_tokens.shape

    tails = np.asarray(edge_index[1], dtype=np.int64)
    cvt = np.asarray(node_is_cvt, dtype=bool)
    cvt_nodes = np.nonzero(cvt)[0]                      # sorted CVT node ids
    ncvt = len(cvt_nodes)

    eids = np.nonzero(cvt[tails])[0]                    # contributing edges
    et = tails[eids]
    order = np.argsort(et, kind="stable")
    eids = eids[order]
    et = et[order]

    # per-CVT-node edge counts (aligned with cvt_nodes order)
    cnt_per_node = np.bincount(et, minlength=n_nodes)[cvt_nodes]
    assert cnt_per_node.max() <= P, "node with >128 edges unsupported"

    # split CVT nodes into 8 contiguous equal chunks
    bounds = [round(ncvt * c / n_cores) for c in range(n_cores + 1)]

    # greedy-pack each core's nodes into windows (<=128 nodes, <=128 edges)
    win = np.empty(ncvt, np.int64)
    seg = np.empty(ncvt, np.int64)
    estart = np.empty(ncvt, np.int64)
    Ws = []
    for c in range(n_cores):
        lo, hi = bounds[c], bounds[c + 1]
        w = 0
        nodes_in = 0
        edges_in = 0
        for i in range(lo, hi):
            k = cnt_per_node[i]
            if nodes_in == P or edges_in + k > P:
                w += 1
                nodes_in = 0
                edges_in = 0
            win[i] = w
            seg[i] = nodes_in
            estart[i] = edges_in
            nodes_in += 1
            edges_in += k
        Ws.append(w + 1 if hi > lo else 0)
    W = max(1, max(Ws))
    W = ((W + G - 1) // G) * G
    WG = W // G

    # per-edge window/slot (edges are sorted by tail; node rank via cumsum)
    node_rank_of_edge = np.searchsorted(cvt_nodes, et)   # index into cvt arrays
    first_edge_of_node = np.concatenate(
        [[0], np.cumsum(cnt_per_node)[:-1]]
    )
    rank_in_node = np.arange(len(et)) - first_edge_of_node[node_rank_of_edge]
    e_win = win[node_rank_of_edge]
    e_slot = estart[node_rank_of_edge] + rank_in_node
    core_of_node = np.searchsorted(bounds, np.arange(ncvt), side="right") - 1
    e_core = core_of_node[node_rank_of_edge]

    # edge features, routed: Xe_pad[core, w, slot] = [rel[e] | nod[e]]
    X = np.concatenate(
        [relation_tokens[eids], node_tokens[eids]], axis=1
    )                                                    # [ne, 2H] f32
    Xe = np.zeros((n_cores, W, P, 2 * hid), np.float32)
    Xe[e_core, e_win, e_slot] = X

    # one-hot seg matrix per window: ohw[slot, n] = 1 if slot's tail is
    # local node n (pad slots = all-zero rows)
    segf = np.full((n_cores, W, P), -1, np.int32)
    segf[e_core, e_win, e_slot] = seg[node_rank_of_edge].astype(np.int32)
    ohw = (segf[..., None] == np.arange(P, dtype=np.int32)
           ).astype(np.float32)                          # [C, W, P, P]

    # xg chunk layout per window: 4 X^T chunks [feat, slot] + oh [slot, n]
    xtc = (Xe.reshape(n_cores, W, P, 4, P)
           .transpose(0, 1, 4, 3, 2))                    # [C, W, p, c, slot]
    blk = np.concatenate(
        [xtc.reshape(n_cores, W, P, 4 * P), ohw], axis=3
    )                                                    # [C, W, P, 5*P]
    xt_all = (
        blk.reshape(n_cores, WG, G, P, CHW * P)
        .transpose(0, 1, 3, 2, 4)
        .reshape(n_cores, WG, P, G * CHW * P)
        .astype(ml_dtypes.bfloat16)
    )

    # weights: wch[p, c*WAUG + h] = W_msg[h, c*128+p]; col 256 = a_eff
    a_eff = (attn_vector.astype(np.float64) @ np.asarray(W_msg, np.float64)
             ).astype(np.float32)                        # [2H]
    Wt = np.asarray(W_msg, np.float32).T                 # [2H, H]
    wch = np.zeros((P, 4 * WAUG), np.float32)
    for c in range(4):
        wch[:, c * WAUG:c * WAUG + hid] = Wt[c * P:(c + 1) * P, :]
        wch[:, c * WAUG + hid] = a_eff[c * P:(c + 1) * P]
    wch = wch.astype(ml_dtypes.bfloat16)

    in_maps = [
        {"xt": xt_all[c], "wch": wch}
        for c in range(n_cores)
    ]
    # scatter: dev_out[core] reshaped [W*P, WAUG] row (w*P + seg) -> node id
    scatter = []
    for c in range(n_cores):
        lo, hi = bounds[c], bounds[c + 1]
        rows = win[lo:hi] * P + seg[lo:hi]
        scatter.append((cvt_nodes[lo:hi], rows))
    return in_maps, W, scatter


def kernel(**inputs) -> np.ndarray:
    from concourse import bass2jax

    node_tokens = np.asarray(inputs["node_tokens"], np.float32)
    in_maps, W, scatter = _host_prep(
        node_tokens,
        inputs["relation_tokens"],
        inputs["edge_index"],
        inputs["node_is_cvt"],
        inputs["shared_cvt"],
        inputs["attn_vector"],
        inputs["W_msg"],
    )
    nc = _PROGRAM_CACHE.get(W)
    if nc is None:
        nc = _build_program(W)
        _PROGRAM_CACHE[W] = nc
    results = bass2jax.run_bass_via_pjrt(nc, in_maps, n_cores=len(in_maps))
    hid = node_tokens.shape[1]
    shared = np.asarray(inputs["shared_cvt"], np.float32)
    out_full = node_tokens.copy()
    for c, r in enumerate(results):
        WG = r["out"].shape[0]
        dev = (np.asarray(r["out"]).astype(np.float32)
               .reshape(WG, P, G, WAUG)
               .transpose(0, 2, 1, 3).reshape(-1, WAUG))  # [W*P, WAUG]
        node_ids, rows = scatter[c]
        sel = dev[rows]
        agg = sel[:, :hid]
        den = np.maximum(sel[:, hid], 1e-30)[:, None]
        out_full[node_ids] = agg / den + shared
    return out_full
